# revision 48
# baseline (speedup 1.0000x reference)
"""Two-layer GCN (PyG GCNConv x2, relu between) on 8 trn2 NeuronCores.

Strategy (dst-node partitioned, all on-device math):
  - Nodes are sharded across 8 cores by destination row (12500/core).
  - Layer tables (dinv * (x@W1), then (dinv*relu(h))@W2-space inputs) are
    computed shard-wise on-device, AllGather'ed into a replicated DRAM table
    of 256B rows, and per-edge messages are fetched with GPSIMD dma_gather.
  - Segment-sum per 128-dst block is a PE matmul with a one-hot selection
    matrix built on DVE via is_equal against an iota row; PSUM accumulates
    across message chunks, so no scatter is needed.
  - Epilogues apply dinv/bias/relu and chain directly into the next layer's
    table transform. Final output is produced transposed and unsharded on
    host.

The Bass program is identical on all cores (SPMD); per-(block, src-group)
chunk counts are the max over cores, with padding slots pointing at a zero
table row.
"""

import math
import sys

sys.path.insert(0, "/opt/trn_rl_repo")

import numpy as np


# ---------------------------------------------------------------------------
# configuration
# ---------------------------------------------------------------------------
class Cfg:
    CORES = 8
    N = 100000
    IN_C = 128
    HID = 64
    OUT_C = 40
    NPC = 12500  # nodes per core
    NPC_PAD = 12544  # = 98 * 128
    BLK = 128
    SBB = 4  # dst blocks per superblock (gather-call granularity)
    # int16 unsigned-use reach: the gather ucode treats indices as unsigned
    # in the address math, so only [0, 32767] is usable per window.
    GROUP_ROWS = 32768

    def group_bias(self, g):
        return 0
    SG = 8  # chunks per S-build op
    MAXCH = 8  # max chunks per dma_gather call
    NQ = 3  # SWDGE queues used round-robin

    @property
    def NBLK(self):
        return self.NPC_PAD // self.BLK

    @property
    def NSB(self):
        return math.ceil(self.NBLK / self.SBB)

    @property
    def TAB(self):
        return self.NPC_PAD * self.CORES

    @property
    def NGRP(self):
        return math.ceil(self.TAB / self.GROUP_ROWS)


# ---------------------------------------------------------------------------
# host-side prep: shard edges, build shared static schedule + per-core arrays
# ---------------------------------------------------------------------------
def _prepare(cfg, edge_index):
    src = np.asarray(edge_index[0], dtype=np.int64)
    dst = np.asarray(edge_index[1], dtype=np.int64)
    # deg includes the self-loop; the loop edge itself is applied in the
    # on-device epilogue (adds dinv[d]*table_row[d]), not gathered.
    deg = (np.bincount(dst, minlength=cfg.N) + 1.0).astype(np.float32)

    # zero table row per src-group (core pad rows are zero in both tables).
    # Must sit in the biased-nonnegative upper half of the group so sorted
    # chunks end with a nonnegative index (ucode trims trailing negatives).
    zrow = []
    for g in range(cfg.NGRP):
        lo = g * cfg.GROUP_ROWS + cfg.group_bias(g)
        hi = min((g + 1) * cfg.GROUP_ROWS, cfg.TAB)
        r = None
        for c in range(cfg.CORES):
            p0, p1 = c * cfg.NPC_PAD + cfg.NPC, (c + 1) * cfg.NPC_PAD
            a, b = max(p0, lo), min(p1, hi)
            if a < b:
                r = a
                break
        assert r is not None, f"no zero row available in src-group {g}"
        zrow.append(r)

    owner = dst // cfg.NPC
    dl_all = dst - owner * cfg.NPC
    srow_all = (src // cfg.NPC) * cfg.NPC_PAD + (src % cfg.NPC)
    grp_all = srow_all // cfg.GROUP_ROWS
    blk_all = dl_all // cfg.BLK

    per_core = []
    counts = np.zeros((cfg.CORES, cfg.NBLK, cfg.NGRP), dtype=np.int64)
    for c in range(cfg.CORES):
        m = owner == c
        srow, dl, grp, blk = srow_all[m], dl_all[m], grp_all[m], blk_all[m]
        # emission order: (superblock, group, block)
        key = (blk // cfg.SBB) * (cfg.NGRP * cfg.SBB) + grp * cfg.SBB + (blk % cfg.SBB)
        order = np.argsort(key, kind="stable")
        per_core.append((srow[order], dl[order], key[order]))
        np.add.at(counts[c], (blk, grp), 1)

    sched = np.ceil(counts.max(axis=0) / cfg.BLK).astype(np.int64)  # [NBLK, NGRP]

    # chunk sequence in emission order; calls = one dma_gather per (sb, g)
    chunk_blocks = []  # block id per chunk
    calls = []  # (g, first_chunk, n_chunks) per gather call
    for sb in range(cfg.NSB):
        blo, bhi = sb * cfg.SBB, min((sb + 1) * cfg.SBB, cfg.NBLK)
        for g in range(cfg.NGRP):
            nch = int(sched[blo:bhi, g].sum())
            if nch == 0:
                continue
            # split large calls so one SWDGE op can't overrun the desc ring;
            # split evenly so concurrent queue-pairs finish together (the
            # Pool exec window retires in order, so duration variance idles
            # the other pairs)
            nparts = math.ceil(nch / cfg.MAXCH)
            done = 0
            for p in range(nparts):
                take = (nch - done) // (nparts - p)
                calls.append((g, len(chunk_blocks) + done, take))
                done += take
            for b in range(blo, bhi):
                chunk_blocks.extend([b] * int(sched[b, g]))
    nchunk = len(chunk_blocks)
    nslot = nchunk * cfg.BLK

    # per-core slot arrays following the shared schedule
    idx_maps = []
    dstloc_maps = []
    for c in range(cfg.CORES):
        srow, dl, key = per_core[c]
        idx_arr = np.empty(nslot, dtype=np.int64)
        dloc_arr = np.zeros(nslot, dtype=np.int64)
        pos = 0
        ei = 0
        for sb in range(cfg.NSB):
            blo, bhi = sb * cfg.SBB, min((sb + 1) * cfg.SBB, cfg.NBLK)
            for g in range(cfg.NGRP):
                for b in range(blo, bhi):
                    n = int(counts[c, b, g])
                    cap = int(sched[b, g]) * cfg.BLK
                    bias = g * cfg.GROUP_ROWS + cfg.group_bias(g)
                    idx_arr[pos : pos + n] = srow[ei : ei + n] - bias
                    dloc_arr[pos : pos + n] = dl[ei : ei + n] % cfg.BLK
                    idx_arr[pos + n : pos + cap] = zrow[g] - bias
                    # pad dstloc stays 0 (gathers a zero row -> adds nothing)
                    pos += cap
                    ei += n
        assert pos == nslot and ei == len(srow)
        assert idx_arr.min() >= 0 and idx_arr.max() < cfg.GROUP_ROWS
        # sort each 128-slot chunk ascending: gather addresses become
        # monotone within a chunk, which helps DMA read locality.
        ic = idx_arr.reshape(-1, cfg.BLK)
        dc = dloc_arr.reshape(-1, cfg.BLK)
        order2 = np.argsort(ic, axis=1, kind="stable")
        idx_arr = np.take_along_axis(ic, order2, axis=1).reshape(-1)
        dloc_arr = np.take_along_axis(dc, order2, axis=1).reshape(-1)
        import ml_dtypes

        idx_maps.append(np.tile(idx_arr.astype(np.int16).reshape(-1, 16).T, (8, 1)))
        dstloc_maps.append(
            np.ascontiguousarray(
                dloc_arr.astype(ml_dtypes.bfloat16).reshape(nchunk, cfg.BLK).T
            )
        )

    return {
        "deg": deg,
        "sched": sched,
        "chunk_blocks": chunk_blocks,
        "calls": calls,
        "nchunk": nchunk,
        "nslot": nslot,
        "idx_maps": idx_maps,
        "dstloc_maps": dstloc_maps,
    }


# ---------------------------------------------------------------------------
# device program
# ---------------------------------------------------------------------------
def _build(cfg, chunk_blocks, calls, debug=False, taps=False):
    import concourse.bacc as bacc
    import concourse.mybir as mybir
    import concourse.tile as tile
    from concourse import library_config

    fp32 = mybir.dt.float32
    bf16 = mybir.dt.bfloat16
    AF = mybir.ActivationFunctionType
    ALU = mybir.AluOpType

    nchunk = len(chunk_blocks)
    # first/last chunk per block (for PSUM start/stop flags)
    first_chunk = {}
    last_chunk = {}
    for j, b in enumerate(chunk_blocks):
        first_chunk.setdefault(b, j)
        last_chunk[b] = j
    max_call_ch = max(n for _, _, n in calls)

    nc = bacc.Bacc(
        "TRN2", target_bir_lowering=False, debug=debug, num_swdge_queues=4
    )

    xT_in = nc.dram_tensor("xT", [cfg.IN_C, cfg.NPC_PAD], fp32, kind="ExternalInput")
    W1_in = nc.dram_tensor("W1", [cfg.IN_C, cfg.HID], fp32, kind="ExternalInput")
    W2p_in = nc.dram_tensor("W2p", [cfg.HID, cfg.HID], bf16, kind="ExternalInput")
    b1_in = nc.dram_tensor("b1c", [cfg.HID, 1], fp32, kind="ExternalInput")
    b2_in = nc.dram_tensor("b2c", [cfg.OUT_C, 1], fp32, kind="ExternalInput")
    dinvnw_in = nc.dram_tensor(
        "dinv_nw", [cfg.BLK, cfg.NBLK], fp32, kind="ExternalInput"
    )
    dinvfl_in = nc.dram_tensor(
        "dinv_flat", [1, cfg.NPC_PAD], fp32, kind="ExternalInput"
    )
    idx_in = nc.dram_tensor(
        "idxs", [128, (nchunk * cfg.BLK) // 16, ], mybir.dt.int16, kind="ExternalInput"
    )
    dstloc_in = nc.dram_tensor(
        "dstloc", [cfg.BLK, nchunk], bf16, kind="ExternalInput"
    )
    out_t = nc.dram_tensor(
        "outT", [cfg.OUT_C, cfg.NPC_PAD], fp32, kind="ExternalOutput"
    )
    # table rows are 128 bf16 = 256B (the dma_gather granularity floor);
    # only the first HID columns carry data, the rest is never read.
    TW = 128
    shard1 = nc.dram_tensor("shard1", [cfg.NPC_PAD, TW], bf16)
    shard2 = nc.dram_tensor("shard2", [cfg.NPC_PAD, TW], bf16)
    table1 = nc.dram_tensor(
        "table1", [cfg.TAB, TW], bf16, addr_space="Shared"
    )
    table2 = nc.dram_tensor(
        "table2", [cfg.TAB, TW], bf16, addr_space="Shared"
    )
    import ml_dtypes

    iota_c = nc.inline_tensor(
        np.tile(np.arange(cfg.BLK, dtype=np.float32), (128, cfg.SG))
        .reshape(128, cfg.SG * cfg.BLK)
        .astype(ml_dtypes.bfloat16),
        name="iota_sg",
    )
    eye_c = nc.inline_tensor(
        np.eye(cfg.BLK, dtype=ml_dtypes.bfloat16), name="eye128"
    )
    ones_c = nc.inline_tensor(
        np.ones((1, cfg.HID), dtype=np.float32), name="ones64"
    )

    replica = [list(range(cfg.CORES))]

    with tile.TileContext(nc) as tc:
        with (
            tc.tile_pool(name="cst", bufs=1) as cst,
            tc.tile_pool(name="gp", bufs=16) as gp,
            tc.tile_pool(name="sp", bufs=8) as sp,
            tc.tile_pool(name="dv", bufs=2) as dv,
            tc.tile_pool(name="ev", bufs=2) as ev,
            tc.tile_pool(name="stg", bufs=1) as stg,
        ):
            nc.gpsimd.load_library(library_config.mlp)

            # ---- constants ----
            W1t = cst.tile([cfg.IN_C, cfg.HID], fp32)
            nc.sync.dma_start(W1t[:], W1_in[:])
            W2t = cst.tile([cfg.HID, cfg.HID], bf16)
            nc.sync.dma_start(W2t[:], W2p_in[:])
            b1t = cst.tile([cfg.HID, 1], fp32)
            nc.sync.dma_start(b1t[:], b1_in[:])
            b2t = cst.tile([cfg.OUT_C, 1], fp32)
            nc.sync.dma_start(b2t[:], b2_in[:])
            iota = cst.tile([128, cfg.SG * cfg.BLK], bf16)
            nc.sync.dma_start(iota[:], iota_c[:])
            eye = cst.tile([cfg.BLK, cfg.BLK], bf16)
            nc.sync.dma_start(eye[:], eye_c[:])
            ones64 = cst.tile([1, cfg.HID], fp32)
            nc.sync.dma_start(ones64[:], ones_c[:])
            dinvnw = cst.tile([cfg.BLK, cfg.NBLK], fp32)
            nc.sync.dma_start(dinvnw[:], dinvnw_in[:])
            idxt = cst.tile([128, (nchunk * cfg.BLK) // 16], mybir.dt.int16)
            nc.sync.dma_start(idxt[:], idx_in[:])
            dstloct = cst.tile([cfg.BLK, nchunk], bf16)
            nc.sync.dma_start(dstloct[:], dstloc_in[:])
            # dinv replicated across HID partitions for the whole shard,
            # built once with K=1 ones outer-products on the PE
            dinvw = cst.tile([cfg.HID, cfg.NPC_PAD], bf16)
            DW = 512
            with (
                tc.tile_pool(name="dfl", bufs=1) as dfl,
                tc.tile_pool(name="psdw", bufs=2, space="PSUM") as psdw,
            ):
                dinvfl = dfl.tile([1, cfg.NPC_PAD], fp32)
                nc.sync.dma_start(dinvfl[:], dinvfl_in[:])
                for t in range(0, cfg.NPC_PAD, DW):
                    w = min(DW, cfg.NPC_PAD - t)
                    pd = psdw.tile([cfg.HID, DW], fp32, tag="pd")
                    nc.tensor.matmul(
                        pd[:, :w],
                        lhsT=ones64[:],
                        rhs=dinvfl[:, t : t + w],
                        start=True,
                        stop=True,
                    )
                    nc.scalar.activation(dinvw[:, t : t + w], pd[:, :w], AF.Copy)

            # staging for both layer tables, kept resident: stag rows double
            # as the self-loop contribution (accumulated via transpose-matmul)
            stag1 = stg.tile([cfg.BLK, cfg.NBLK, cfg.HID], bf16)
            stag2 = stg.tile([cfg.BLK, cfg.NBLK, cfg.HID], bf16)

            # ---- layer-1 transform: shard1 = dinv * (x @ W1), row-major ----
            with (
                tc.tile_pool(name="phA", bufs=3) as pa,
                tc.tile_pool(name="psA", bufs=4, space="PSUM") as psA,
            ):
                for b in range(cfg.NBLK):
                    xc = pa.tile([cfg.IN_C, cfg.BLK], fp32)
                    nc.sync.dma_start(
                        xc[:], xT_in[:, b * cfg.BLK : (b + 1) * cfg.BLK]
                    )
                    ps = psA.tile([cfg.BLK, cfg.HID], fp32)
                    nc.tensor.matmul(ps[:], lhsT=xc[:], rhs=W1t[:], start=True, stop=True)
                    nc.vector.tensor_scalar(
                        out=stag1[:, b, :],
                        in0=ps[:],
                        scalar1=dinvnw[:, b : b + 1],
                        scalar2=None,
                        op0=ALU.mult,
                    )
                nc.sync.dma_start(
                    shard1.rearrange("(b p) d -> p b d", p=cfg.BLK)[
                        :, :, : cfg.HID
                    ],
                    stag1[:],
                )

            nc.gpsimd.collective_compute(
                "AllGather",
                mybir.AluOpType.bypass,
                replica_groups=replica,
                ins=[shard1[:]],
                outs=[table1[:]],
            )

            # ---- aggregation layer (shared for both layers) ----
            def agg_layer(layer, table):
                ch_out = cfg.HID if layer == 1 else cfg.OUT_C
                # S tiles for the whole chunk sequence, built in groups of SG
                s_tiles = {}

                def s_for(j):
                    gi = j // cfg.SG
                    if gi not in s_tiles:
                        n = min(cfg.SG, nchunk - gi * cfg.SG)
                        st = sp.tile([128, cfg.SG * cfg.BLK], bf16, tag="s")
                        nc.vector.tensor_tensor(
                            out=st[:].rearrange("p (a b) -> p a b", b=cfg.BLK)[
                                :, :n, :
                            ],
                            in0=iota[:].rearrange("p (a b) -> p a b", b=cfg.BLK)[
                                :, :n, :
                            ],
                            in1=dstloct[:, gi * cfg.SG : gi * cfg.SG + n].to_broadcast(
                                [128, n, cfg.BLK]
                            ),
                            op=ALU.is_equal,
                        )
                        s_tiles[gi] = st
                    return s_tiles[gi], (j % cfg.SG)

                ci = 0  # call cursor
                SBW = cfg.SBB * cfg.BLK
                stag_self = stag1 if layer == 1 else stag2
                # last chunk per superblock (the bank-wide accumulation group
                # gets exactly one start and one stop)
                last_j_sb = {}
                for j, b in enumerate(chunk_blocks):
                    last_j_sb[b // cfg.SBB] = j
                for sb in range(cfg.NSB):
                    blo, bhi = sb * cfg.SBB, min((sb + 1) * cfg.SBB, cfg.NBLK)
                    nsb = (bhi - blo) * cfg.BLK
                    ssl = slice(blo * cfg.BLK, blo * cfg.BLK + nsb)

                    # one PSUM accumulator (= one 2KB zero region) spanning the
                    # superblock; the first matmul's start marks the whole bank
                    # pending-zero, so each slice's first write overwrites.
                    # Seed each block's slice with its self-loop contribution
                    # (stag block transposed via identity matmul).
                    pssb = tc_psum.tile([ch_out, SBW], fp32, tag=f"ps{layer}")
                    for b in range(blo, bhi):
                        off = (b - blo) * cfg.BLK
                        nc.tensor.matmul(
                            pssb[:, off : off + cfg.BLK],
                            lhsT=stag_self[:, b, :ch_out],
                            rhs=eye[:],
                            start=(b == blo),
                            stop=False,
                        )

                    # gather + matmul-aggregate this superblock's calls
                    while ci < len(calls):
                        g, j0, nch = calls[ci]
                        if chunk_blocks[j0] >= bhi:
                            break
                        qn = ci % cfg.NQ
                        ci += 1
                        base = g * cfg.GROUP_ROWS + cfg.group_bias(g)
                        rows = min(cfg.GROUP_ROWS, cfg.TAB - base)
                        gt = gp.tile([128, max_call_ch, TW], bf16, tag="g")
                        nc.gpsimd.dma_gather(
                            gt[:, :nch, :],
                            table[base : base + rows, :],
                            idxt[:, (j0 * cfg.BLK) // 16 : ((j0 + nch) * cfg.BLK) // 16],
                            nch * cfg.BLK,
                            nch * cfg.BLK,
                            TW,
                            queue_num=qn,
                        )
                        for j in range(j0, j0 + nch):
                            b = chunk_blocks[j]
                            off = (b - blo) * cfg.BLK
                            st, k = s_for(j)
                            nc.tensor.matmul(
                                pssb[:, off : off + cfg.BLK],
                                lhsT=gt[:, j - j0, :ch_out],
                                rhs=st[:, k * cfg.BLK : (k + 1) * cfg.BLK],
                                start=False,
                                stop=(j == last_j_sb[sb]),
                            )

                    # superblock-wide epilogue
                    if layer == 1:
                        t1 = ev.tile([cfg.HID, SBW], fp32, tag="t1")
                        nc.vector.tensor_tensor(
                            out=t1[:, :nsb],
                            in0=pssb[:, :nsb],
                            in1=dinvw[:, ssl],
                            op=ALU.mult,
                        )
                        nc.scalar.activation(
                            t1[:, :nsb], t1[:, :nsb], AF.Relu, bias=b1t[:]
                        )
                        gbt = ev.tile([cfg.HID, SBW], bf16, tag="gb")
                        nc.vector.tensor_tensor(
                            out=gbt[:, :nsb],
                            in0=t1[:, :nsb],
                            in1=dinvw[:, ssl],
                            op=ALU.mult,
                        )
                        for b in range(blo, bhi):
                            off = (b - blo) * cfg.BLK
                            ps2 = tc_ps2.tile([cfg.BLK, cfg.HID], fp32, tag="ps2")
                            nc.tensor.matmul(
                                ps2[:],
                                lhsT=gbt[:, off : off + cfg.BLK],
                                rhs=W2t[:],
                                start=True,
                                stop=True,
                            )
                            nc.vector.tensor_copy(stag2[:, b, :], ps2[:])
                    else:
                        t1 = ev.tile([cfg.HID, SBW], fp32, tag="t1")
                        nc.vector.tensor_tensor(
                            out=t1[: cfg.OUT_C, :nsb],
                            in0=pssb[:, :nsb],
                            in1=dinvw[: cfg.OUT_C, ssl],
                            op=ALU.mult,
                        )
                        nc.vector.tensor_scalar(
                            out=t1[: cfg.OUT_C, :nsb],
                            in0=t1[: cfg.OUT_C, :nsb],
                            scalar1=b2t[:],
                            scalar2=None,
                            op0=ALU.add,
                        )
                        nc.sync.dma_start(out_t[:, ssl], t1[: cfg.OUT_C, :nsb])

            # layer 1 aggregation (+ table2 transform fused in epilogue)
            with (
                tc.tile_pool(name="ps2p", bufs=2, space="PSUM") as tc_ps2,
                tc.tile_pool(name="psagg1", bufs=1, space="PSUM") as tc_psum,
            ):
                agg_layer(1, table1)
                nc.sync.dma_start(
                    shard2.rearrange("(b p) d -> p b d", p=cfg.BLK)[
                        :, :, : cfg.HID
                    ],
                    stag2[:],
                )

            nc.gpsimd.collective_compute(
                "AllGather",
                mybir.AluOpType.bypass,
                replica_groups=replica,
                ins=[shard2[:]],
                outs=[table2[:]],
            )

            # layer 2 aggregation -> transposed output, written per superblock
            with (
                tc.tile_pool(name="psagg2", bufs=1, space="PSUM") as tc_psum,
            ):
                agg_layer(2, table2)

    nc.compile()
    return nc


# ---------------------------------------------------------------------------
# public entry point
# ---------------------------------------------------------------------------
def _make_in_maps(cfg, prep, x, W1, b1, W2, b2):
    import ml_dtypes

    W2p = np.zeros((cfg.HID, cfg.HID), np.float32)
    W2p[:, : cfg.OUT_C] = W2
    W2p = W2p.astype(ml_dtypes.bfloat16)
    deg = prep["deg"]
    in_maps = []
    for c in range(cfg.CORES):
        xs = x[c * cfg.NPC : (c + 1) * cfg.NPC]  # [NPC, IN_C]
        xT = np.zeros((cfg.IN_C, cfg.NPC_PAD), np.float32)
        xT[:, : cfg.NPC] = xs.T
        # pad nodes: dinv = 0 -> pad table rows and outputs are exactly 0
        dinv = np.zeros(cfg.NPC_PAD, np.float32)
        dinv[: cfg.NPC] = 1.0 / np.sqrt(deg[c * cfg.NPC : (c + 1) * cfg.NPC])
        dinv_nw = np.ascontiguousarray(dinv.reshape(cfg.NBLK, cfg.BLK).T)
        in_maps.append(
            {
                "xT": xT,
                "W1": np.asarray(W1, np.float32),
                "W2p": W2p,
                "b1c": np.asarray(b1, np.float32).reshape(cfg.HID, 1),
                "b2c": np.asarray(b2, np.float32).reshape(cfg.OUT_C, 1),
                "dinv_nw": dinv_nw,
                "dinv_flat": dinv.reshape(1, cfg.NPC_PAD),
                "idxs": prep["idx_maps"][c],
                "dstloc": prep["dstloc_maps"][c],
            }
        )
    return in_maps


def _run(cfg, inputs, mode="hw", trace=False, taps=False):
    x = np.asarray(inputs["x"], np.float32)
    edge_index = np.asarray(inputs["edge_index"])
    W1 = np.asarray(inputs["W1"], np.float32)
    b1 = np.asarray(inputs["b1"], np.float32)
    W2 = np.asarray(inputs["W2"], np.float32)
    b2 = np.asarray(inputs["b2"], np.float32)

    prep = _prepare(cfg, edge_index)
    nc = _build(cfg, prep["chunk_blocks"], prep["calls"], debug=(mode == "sim"), taps=taps)
    in_maps = _make_in_maps(cfg, prep, x, W1, b1, W2, b2)

    info = {}
    if mode == "sim":
        from concourse.bass_interp import MultiCoreSim

        sim = MultiCoreSim(nc, cfg.CORES)
        for c in range(cfg.CORES):
            for k, v in in_maps[c].items():
                sim.cores[c].tensor(k)[:] = v
        sim.simulate()
        outs = [sim.cores[c].tensor("outT").copy() for c in range(cfg.CORES)]
    else:
        import concourse.bass_utils as bu

        if trace:
            # avoid the S3 artifact upload in the profile path
            bu.upload_artifacts = lambda d: "(local)"
        r = bu.run_bass_kernel_spmd(
            nc, in_maps, list(range(cfg.CORES)), trace=trace,
            tmpdir=(inputs.get("_tracedir") if trace else None),
        )
        info["exec_time_ns"] = r.exec_time_ns
        info["mean_exec_time_ns"] = r.mean_exec_time_ns
        outs = [r.results[c]["outT"] for c in range(cfg.CORES)]

    out = np.concatenate([o[:, : cfg.NPC].T for o in outs], axis=0)
    return out.astype(np.float32), info


def kernel(**inputs):
    out, _ = _run(Cfg(), inputs, mode="hw")
    return out



# revision 52
# speedup vs baseline: 1.1664x; 1.1664x over previous
"""Two-layer GCN (PyG GCNConv x2, relu between) on 8 trn2 NeuronCores.

Strategy (dst-node partitioned, all on-device math):
  - Nodes are sharded across 8 cores by destination row (12500/core).
  - Layer tables (dinv * (x@W1), then (dinv*relu(h))@W2-space inputs) are
    computed shard-wise on-device, AllGather'ed into a replicated DRAM table
    of 256B rows, and per-edge messages are fetched with GPSIMD dma_gather.
  - Segment-sum per 128-dst block is a PE matmul with a one-hot selection
    matrix built on DVE via is_equal against an iota row; PSUM accumulates
    across message chunks, so no scatter is needed.
  - Epilogues apply dinv/bias/relu and chain directly into the next layer's
    table transform. Final output is produced transposed and unsharded on
    host.

The Bass program is identical on all cores (SPMD); per-(block, src-group)
chunk counts are the max over cores, with padding slots pointing at a zero
table row.
"""

import math
import sys

sys.path.insert(0, "/opt/trn_rl_repo")

import numpy as np


# ---------------------------------------------------------------------------
# configuration
# ---------------------------------------------------------------------------
class Cfg:
    CORES = 8
    N = 100000
    IN_C = 128
    HID = 64
    OUT_C = 40
    NPC = 12500  # nodes per core
    NPC_PAD = 12544  # = 98 * 128
    BLK = 128
    SBB = 4  # dst blocks per superblock (gather-call granularity)
    # int16 unsigned-use reach: the gather ucode treats indices as unsigned
    # in the address math, so only [0, 32767] is usable per window.
    GROUP_ROWS = 32768

    def group_bias(self, g):
        return 0
    SG = 8  # chunks per S-build op
    MAXCH = 8  # max chunks per dma_gather call
    NQ = 3  # SWDGE queues used round-robin

    @property
    def NBLK(self):
        return self.NPC_PAD // self.BLK

    @property
    def NSB(self):
        return math.ceil(self.NBLK / self.SBB)

    @property
    def TAB(self):
        return self.NPC_PAD * self.CORES

    @property
    def NGRP(self):
        return math.ceil(self.TAB / self.GROUP_ROWS)


# ---------------------------------------------------------------------------
# host-side prep: shard edges, build shared static schedule + per-core arrays
# ---------------------------------------------------------------------------
def _prepare(cfg, edge_index):
    src = np.asarray(edge_index[0], dtype=np.int64)
    dst = np.asarray(edge_index[1], dtype=np.int64)
    # deg includes the self-loop; the loop edge itself is applied in the
    # on-device epilogue (adds dinv[d]*table_row[d]), not gathered.
    deg = (np.bincount(dst, minlength=cfg.N) + 1.0).astype(np.float32)

    # zero table row per src-group (core pad rows are zero in both tables).
    # Must sit in the biased-nonnegative upper half of the group so sorted
    # chunks end with a nonnegative index (ucode trims trailing negatives).
    zrow = []
    for g in range(cfg.NGRP):
        lo = g * cfg.GROUP_ROWS + cfg.group_bias(g)
        hi = min((g + 1) * cfg.GROUP_ROWS, cfg.TAB)
        r = None
        for c in range(cfg.CORES):
            p0, p1 = c * cfg.NPC_PAD + cfg.NPC, (c + 1) * cfg.NPC_PAD
            a, b = max(p0, lo), min(p1, hi)
            if a < b:
                r = a
                break
        assert r is not None, f"no zero row available in src-group {g}"
        zrow.append(r)

    owner = dst // cfg.NPC
    dl_all = dst - owner * cfg.NPC
    srow_all = (src // cfg.NPC) * cfg.NPC_PAD + (src % cfg.NPC)
    grp_all = srow_all // cfg.GROUP_ROWS
    blk_all = dl_all // cfg.BLK

    per_core = []
    counts = np.zeros((cfg.CORES, cfg.NBLK, cfg.NGRP), dtype=np.int64)
    for c in range(cfg.CORES):
        m = owner == c
        srow, dl, grp, blk = srow_all[m], dl_all[m], grp_all[m], blk_all[m]
        # emission order: (superblock, group, block)
        key = (blk // cfg.SBB) * (cfg.NGRP * cfg.SBB) + grp * cfg.SBB + (blk % cfg.SBB)
        order = np.argsort(key, kind="stable")
        per_core.append((srow[order], dl[order], key[order]))
        np.add.at(counts[c], (blk, grp), 1)

    sched = np.ceil(counts.max(axis=0) / cfg.BLK).astype(np.int64)  # [NBLK, NGRP]

    # chunk sequence in emission order; calls = one dma_gather per (sb, g)
    chunk_blocks = []  # block id per chunk
    calls = []  # (g, first_chunk, n_chunks) per gather call
    for sb in range(cfg.NSB):
        blo, bhi = sb * cfg.SBB, min((sb + 1) * cfg.SBB, cfg.NBLK)
        for g in range(cfg.NGRP):
            nch = int(sched[blo:bhi, g].sum())
            if nch == 0:
                continue
            # split large calls so one SWDGE op can't overrun the desc ring;
            # split evenly so concurrent queue-pairs finish together (the
            # Pool exec window retires in order, so duration variance idles
            # the other pairs)
            nparts = math.ceil(nch / cfg.MAXCH)
            done = 0
            for p in range(nparts):
                take = (nch - done) // (nparts - p)
                calls.append((g, len(chunk_blocks) + done, take))
                done += take
            for b in range(blo, bhi):
                chunk_blocks.extend([b] * int(sched[b, g]))
    nchunk = len(chunk_blocks)
    nslot = nchunk * cfg.BLK

    # per-core slot arrays following the shared schedule
    idx_maps = []
    dstloc_maps = []
    for c in range(cfg.CORES):
        srow, dl, key = per_core[c]
        idx_arr = np.empty(nslot, dtype=np.int64)
        dloc_arr = np.zeros(nslot, dtype=np.int64)
        pos = 0
        ei = 0
        for sb in range(cfg.NSB):
            blo, bhi = sb * cfg.SBB, min((sb + 1) * cfg.SBB, cfg.NBLK)
            for g in range(cfg.NGRP):
                for b in range(blo, bhi):
                    n = int(counts[c, b, g])
                    cap = int(sched[b, g]) * cfg.BLK
                    bias = g * cfg.GROUP_ROWS + cfg.group_bias(g)
                    idx_arr[pos : pos + n] = srow[ei : ei + n] - bias
                    dloc_arr[pos : pos + n] = dl[ei : ei + n] % cfg.BLK
                    idx_arr[pos + n : pos + cap] = zrow[g] - bias
                    # pad dstloc stays 0 (gathers a zero row -> adds nothing)
                    pos += cap
                    ei += n
        assert pos == nslot and ei == len(srow)
        assert idx_arr.min() >= 0 and idx_arr.max() < cfg.GROUP_ROWS
        # sort each 128-slot chunk ascending: gather addresses become
        # monotone within a chunk, which helps DMA read locality.
        ic = idx_arr.reshape(-1, cfg.BLK)
        dc = dloc_arr.reshape(-1, cfg.BLK)
        order2 = np.argsort(ic, axis=1, kind="stable")
        idx_arr = np.take_along_axis(ic, order2, axis=1).reshape(-1)
        dloc_arr = np.take_along_axis(dc, order2, axis=1).reshape(-1)
        import ml_dtypes

        idx_maps.append(np.tile(idx_arr.astype(np.int16).reshape(-1, 16).T, (8, 1)))
        dstloc_maps.append(
            np.ascontiguousarray(
                dloc_arr.astype(ml_dtypes.bfloat16).reshape(nchunk, cfg.BLK).T
            )
        )

    return {
        "deg": deg,
        "sched": sched,
        "chunk_blocks": chunk_blocks,
        "calls": calls,
        "nchunk": nchunk,
        "nslot": nslot,
        "idx_maps": idx_maps,
        "dstloc_maps": dstloc_maps,
    }


# ---------------------------------------------------------------------------
# device program
# ---------------------------------------------------------------------------
def _build(cfg, chunk_blocks, calls, debug=False, taps=False):
    import concourse.bacc as bacc
    import concourse.mybir as mybir
    import concourse.tile as tile
    from concourse import library_config

    fp32 = mybir.dt.float32
    bf16 = mybir.dt.bfloat16
    AF = mybir.ActivationFunctionType
    ALU = mybir.AluOpType

    nchunk = len(chunk_blocks)
    # first/last chunk per block (for PSUM start/stop flags)
    first_chunk = {}
    last_chunk = {}
    for j, b in enumerate(chunk_blocks):
        first_chunk.setdefault(b, j)
        last_chunk[b] = j
    max_call_ch = max(n for _, _, n in calls)

    nc = bacc.Bacc(
        "TRN2", target_bir_lowering=False, debug=debug, num_swdge_queues=4
    )

    xT_in = nc.dram_tensor("xT", [cfg.IN_C, cfg.NPC_PAD], fp32, kind="ExternalInput")
    W1_in = nc.dram_tensor("W1", [cfg.IN_C, cfg.HID], fp32, kind="ExternalInput")
    W2p_in = nc.dram_tensor("W2p", [cfg.HID, cfg.HID], bf16, kind="ExternalInput")
    b1_in = nc.dram_tensor("b1c", [cfg.HID, 1], fp32, kind="ExternalInput")
    b2_in = nc.dram_tensor("b2c", [cfg.OUT_C, 1], fp32, kind="ExternalInput")
    dinvnw_in = nc.dram_tensor(
        "dinv_nw", [cfg.BLK, cfg.NBLK], fp32, kind="ExternalInput"
    )
    dinvfl_in = nc.dram_tensor(
        "dinv_flat", [1, cfg.NPC_PAD], fp32, kind="ExternalInput"
    )
    idx_in = nc.dram_tensor(
        "idxs", [128, (nchunk * cfg.BLK) // 16, ], mybir.dt.int16, kind="ExternalInput"
    )
    dstloc_in = nc.dram_tensor(
        "dstloc", [cfg.BLK, nchunk], bf16, kind="ExternalInput"
    )
    out_t = nc.dram_tensor(
        "outT", [cfg.OUT_C, cfg.NPC_PAD], fp32, kind="ExternalOutput"
    )
    # table rows are 128 bf16 = 256B (the dma_gather granularity floor);
    # only the first HID columns carry data, the rest is never read.
    TW = 128
    shard1 = nc.dram_tensor("shard1", [cfg.NPC_PAD, TW], bf16)
    shard2 = nc.dram_tensor("shard2", [cfg.NPC_PAD, TW], bf16)
    table1 = nc.dram_tensor(
        "table1", [cfg.TAB, TW], bf16, addr_space="Shared"
    )
    table2 = nc.dram_tensor(
        "table2", [cfg.TAB, TW], bf16, addr_space="Shared"
    )
    import ml_dtypes

    iota_c = nc.inline_tensor(
        np.tile(np.arange(cfg.BLK, dtype=np.float32), (128, cfg.SG))
        .reshape(128, cfg.SG * cfg.BLK)
        .astype(ml_dtypes.bfloat16),
        name="iota_sg",
    )
    eye_c = nc.inline_tensor(
        np.eye(cfg.BLK, dtype=ml_dtypes.bfloat16), name="eye128"
    )
    ones_c = nc.inline_tensor(
        np.ones((1, cfg.HID), dtype=np.float32), name="ones64"
    )

    replica = [list(range(cfg.CORES))]

    with tile.TileContext(nc) as tc:
        with (
            tc.tile_pool(name="cst", bufs=1) as cst,
            tc.tile_pool(name="gp", bufs=16) as gp,
            tc.tile_pool(name="sp", bufs=8) as sp,
            tc.tile_pool(name="dv", bufs=2) as dv,
            tc.tile_pool(name="ev", bufs=2) as ev,
            tc.tile_pool(name="stg", bufs=1) as stg,
        ):
            nc.gpsimd.load_library(library_config.mlp)

            # ---- constants ----
            W1t = cst.tile([cfg.IN_C, cfg.HID], fp32)
            nc.sync.dma_start(W1t[:], W1_in[:])
            W2t = cst.tile([cfg.HID, cfg.HID], bf16)
            nc.sync.dma_start(W2t[:], W2p_in[:])
            b1t = cst.tile([cfg.HID, 1], fp32)
            nc.sync.dma_start(b1t[:], b1_in[:])
            b2t = cst.tile([cfg.OUT_C, 1], fp32)
            nc.sync.dma_start(b2t[:], b2_in[:])
            iota = cst.tile([128, cfg.SG * cfg.BLK], bf16)
            nc.sync.dma_start(iota[:], iota_c[:])
            eye = cst.tile([cfg.BLK, cfg.BLK], bf16)
            nc.sync.dma_start(eye[:], eye_c[:])
            ones64 = cst.tile([1, cfg.HID], fp32)
            nc.sync.dma_start(ones64[:], ones_c[:])
            dinvnw = cst.tile([cfg.BLK, cfg.NBLK], fp32)
            nc.sync.dma_start(dinvnw[:], dinvnw_in[:])
            idxt = cst.tile([128, (nchunk * cfg.BLK) // 16], mybir.dt.int16)
            nc.sync.dma_start(idxt[:], idx_in[:])
            dstloct = cst.tile([cfg.BLK, nchunk], bf16)
            nc.sync.dma_start(dstloct[:], dstloc_in[:])
            # dinv replicated across HID partitions for the whole shard,
            # built once with K=1 ones outer-products on the PE
            dinvw = cst.tile([cfg.HID, cfg.NPC_PAD], bf16)
            # staging for dinv*relu(h1) across the whole shard (layer-1 out)
            gbw = cst.tile([cfg.HID, cfg.NPC_PAD], bf16)
            DW = 512
            with (
                tc.tile_pool(name="dfl", bufs=1) as dfl,
                tc.tile_pool(name="psdw", bufs=2, space="PSUM") as psdw,
            ):
                dinvfl = dfl.tile([1, cfg.NPC_PAD], fp32)
                nc.sync.dma_start(dinvfl[:], dinvfl_in[:])
                for t in range(0, cfg.NPC_PAD, DW):
                    w = min(DW, cfg.NPC_PAD - t)
                    pd = psdw.tile([cfg.HID, DW], fp32, tag="pd")
                    nc.tensor.matmul(
                        pd[:, :w],
                        lhsT=ones64[:],
                        rhs=dinvfl[:, t : t + w],
                        start=True,
                        stop=True,
                    )
                    nc.scalar.activation(dinvw[:, t : t + w], pd[:, :w], AF.Copy)

            # staging for both layer tables, kept resident: stag rows double
            # as the self-loop contribution (accumulated via transpose-matmul)
            stag1 = stg.tile([cfg.BLK, cfg.NBLK, cfg.HID], bf16)
            stag2 = stg.tile([cfg.BLK, cfg.NBLK, cfg.HID], bf16)

            # ---- layer-1 transform: shard1 = dinv * (x @ W1), row-major ----
            with (
                tc.tile_pool(name="phA", bufs=3) as pa,
                tc.tile_pool(name="psA", bufs=4, space="PSUM") as psA,
            ):
                for b in range(cfg.NBLK):
                    xc = pa.tile([cfg.IN_C, cfg.BLK], fp32)
                    nc.sync.dma_start(
                        xc[:], xT_in[:, b * cfg.BLK : (b + 1) * cfg.BLK]
                    )
                    ps = psA.tile([cfg.BLK, cfg.HID], fp32)
                    nc.tensor.matmul(ps[:], lhsT=xc[:], rhs=W1t[:], start=True, stop=True)
                    nc.vector.tensor_scalar(
                        out=stag1[:, b, :],
                        in0=ps[:],
                        scalar1=dinvnw[:, b : b + 1],
                        scalar2=None,
                        op0=ALU.mult,
                    )
                nc.sync.dma_start(
                    shard1.rearrange("(b p) d -> p b d", p=cfg.BLK)[
                        :, :, : cfg.HID
                    ],
                    stag1[:],
                )

            nc.gpsimd.collective_compute(
                "AllGather",
                mybir.AluOpType.bypass,
                replica_groups=replica,
                ins=[shard1[:]],
                outs=[table1[:]],
            )

            # ---- aggregation layer (shared for both layers) ----
            def agg_layer(layer, table):
                ch_out = cfg.HID if layer == 1 else cfg.OUT_C
                # S tiles for the whole chunk sequence, built in groups of SG
                s_tiles = {}

                def s_for(j):
                    gi = j // cfg.SG
                    if gi not in s_tiles:
                        n = min(cfg.SG, nchunk - gi * cfg.SG)
                        st = sp.tile([128, cfg.SG * cfg.BLK], bf16, tag="s")
                        nc.vector.tensor_tensor(
                            out=st[:].rearrange("p (a b) -> p a b", b=cfg.BLK)[
                                :, :n, :
                            ],
                            in0=iota[:].rearrange("p (a b) -> p a b", b=cfg.BLK)[
                                :, :n, :
                            ],
                            in1=dstloct[:, gi * cfg.SG : gi * cfg.SG + n].to_broadcast(
                                [128, n, cfg.BLK]
                            ),
                            op=ALU.is_equal,
                        )
                        s_tiles[gi] = st
                    return s_tiles[gi], (j % cfg.SG)

                ci = 0  # call cursor
                SBW = cfg.SBB * cfg.BLK
                stag_self = stag1 if layer == 1 else stag2
                # last chunk per superblock (the bank-wide accumulation group
                # gets exactly one start and one stop)
                last_j_sb = {}
                for j, b in enumerate(chunk_blocks):
                    last_j_sb[b // cfg.SBB] = j
                for sb in range(cfg.NSB):
                    blo, bhi = sb * cfg.SBB, min((sb + 1) * cfg.SBB, cfg.NBLK)
                    nsb = (bhi - blo) * cfg.BLK
                    ssl = slice(blo * cfg.BLK, blo * cfg.BLK + nsb)

                    # one PSUM accumulator (= one 2KB zero region) spanning the
                    # superblock; the first matmul's start marks the whole bank
                    # pending-zero, so each slice's first write overwrites.
                    # Seed each block's slice with its self-loop contribution
                    # (stag block transposed via identity matmul).
                    pssb = tc_psum.tile([ch_out, SBW], fp32, tag=f"ps{layer}")
                    for b in range(blo, bhi):
                        off = (b - blo) * cfg.BLK
                        nc.tensor.matmul(
                            pssb[:, off : off + cfg.BLK],
                            lhsT=stag_self[:, b, :ch_out],
                            rhs=eye[:],
                            start=(b == blo),
                            stop=False,
                        )

                    # gather + matmul-aggregate this superblock's calls
                    while ci < len(calls):
                        g, j0, nch = calls[ci]
                        if chunk_blocks[j0] >= bhi:
                            break
                        qn = ci % cfg.NQ
                        ci += 1
                        base = g * cfg.GROUP_ROWS + cfg.group_bias(g)
                        rows = min(cfg.GROUP_ROWS, cfg.TAB - base)
                        gt = gp.tile([128, max_call_ch, TW], bf16, tag="g")
                        nc.gpsimd.dma_gather(
                            gt[:, :nch, :],
                            table[base : base + rows, :],
                            idxt[:, (j0 * cfg.BLK) // 16 : ((j0 + nch) * cfg.BLK) // 16],
                            nch * cfg.BLK,
                            nch * cfg.BLK,
                            TW,
                            queue_num=qn,
                        )
                        for j in range(j0, j0 + nch):
                            b = chunk_blocks[j]
                            off = (b - blo) * cfg.BLK
                            st, k = s_for(j)
                            nc.tensor.matmul(
                                pssb[:, off : off + cfg.BLK],
                                lhsT=gt[:, j - j0, :ch_out],
                                rhs=st[:, k * cfg.BLK : (k + 1) * cfg.BLK],
                                start=False,
                                stop=(j == last_j_sb[sb]),
                            )

                    # superblock-wide epilogue
                    if layer == 1:
                        t1 = ev.tile([cfg.HID, SBW], fp32, tag="t1")
                        nc.vector.tensor_tensor(
                            out=t1[:, :nsb],
                            in0=pssb[:, :nsb],
                            in1=dinvw[:, ssl],
                            op=ALU.mult,
                        )
                        nc.scalar.activation(
                            t1[:, :nsb], t1[:, :nsb], AF.Relu, bias=b1t[:]
                        )
                        # stage dinv*relu(h) for the whole shard; the W2
                        # transform runs after the gather loop so it never
                        # stalls the in-order PE stream mid-layer
                        nc.vector.tensor_tensor(
                            out=gbw[:, ssl],
                            in0=t1[:, :nsb],
                            in1=dinvw[:, ssl],
                            op=ALU.mult,
                        )
                    else:
                        t1 = ev.tile([cfg.HID, SBW], fp32, tag="t1")
                        nc.vector.tensor_tensor(
                            out=t1[: cfg.OUT_C, :nsb],
                            in0=pssb[:, :nsb],
                            in1=dinvw[: cfg.OUT_C, ssl],
                            op=ALU.mult,
                        )
                        nc.vector.tensor_scalar(
                            out=t1[: cfg.OUT_C, :nsb],
                            in0=t1[: cfg.OUT_C, :nsb],
                            scalar1=b2t[:],
                            scalar2=None,
                            op0=ALU.add,
                        )
                        nc.sync.dma_start(out_t[:, ssl], t1[: cfg.OUT_C, :nsb])

            # layer 1 aggregation; the W2 table-2 transform runs as one batch
            # after the gather loop (keeps the PE stream stall-free mid-layer)
            with (
                tc.tile_pool(name="ps2p", bufs=4, space="PSUM") as tc_ps2,
                tc.tile_pool(name="psagg1", bufs=2, space="PSUM") as tc_psum,
            ):
                agg_layer(1, table1)
                for b in range(cfg.NBLK):
                    bsl = slice(b * cfg.BLK, (b + 1) * cfg.BLK)
                    ps2 = tc_ps2.tile([cfg.BLK, cfg.HID], fp32, tag="ps2")
                    nc.tensor.matmul(
                        ps2[:],
                        lhsT=gbw[:, bsl],
                        rhs=W2t[:],
                        start=True,
                        stop=True,
                    )
                    nc.vector.tensor_copy(stag2[:, b, :], ps2[:])
                nc.sync.dma_start(
                    shard2.rearrange("(b p) d -> p b d", p=cfg.BLK)[
                        :, :, : cfg.HID
                    ],
                    stag2[:],
                )

            nc.gpsimd.collective_compute(
                "AllGather",
                mybir.AluOpType.bypass,
                replica_groups=replica,
                ins=[shard2[:]],
                outs=[table2[:]],
            )

            # layer 2 aggregation -> transposed output, written per superblock
            with (
                tc.tile_pool(name="psagg2", bufs=2, space="PSUM") as tc_psum,
            ):
                agg_layer(2, table2)

    nc.compile()
    return nc


# ---------------------------------------------------------------------------
# public entry point
# ---------------------------------------------------------------------------
def _make_in_maps(cfg, prep, x, W1, b1, W2, b2):
    import ml_dtypes

    W2p = np.zeros((cfg.HID, cfg.HID), np.float32)
    W2p[:, : cfg.OUT_C] = W2
    W2p = W2p.astype(ml_dtypes.bfloat16)
    deg = prep["deg"]
    in_maps = []
    for c in range(cfg.CORES):
        xs = x[c * cfg.NPC : (c + 1) * cfg.NPC]  # [NPC, IN_C]
        xT = np.zeros((cfg.IN_C, cfg.NPC_PAD), np.float32)
        xT[:, : cfg.NPC] = xs.T
        # pad nodes: dinv = 0 -> pad table rows and outputs are exactly 0
        dinv = np.zeros(cfg.NPC_PAD, np.float32)
        dinv[: cfg.NPC] = 1.0 / np.sqrt(deg[c * cfg.NPC : (c + 1) * cfg.NPC])
        dinv_nw = np.ascontiguousarray(dinv.reshape(cfg.NBLK, cfg.BLK).T)
        in_maps.append(
            {
                "xT": xT,
                "W1": np.asarray(W1, np.float32),
                "W2p": W2p,
                "b1c": np.asarray(b1, np.float32).reshape(cfg.HID, 1),
                "b2c": np.asarray(b2, np.float32).reshape(cfg.OUT_C, 1),
                "dinv_nw": dinv_nw,
                "dinv_flat": dinv.reshape(1, cfg.NPC_PAD),
                "idxs": prep["idx_maps"][c],
                "dstloc": prep["dstloc_maps"][c],
            }
        )
    return in_maps


def _run(cfg, inputs, mode="hw", trace=False, taps=False):
    x = np.asarray(inputs["x"], np.float32)
    edge_index = np.asarray(inputs["edge_index"])
    W1 = np.asarray(inputs["W1"], np.float32)
    b1 = np.asarray(inputs["b1"], np.float32)
    W2 = np.asarray(inputs["W2"], np.float32)
    b2 = np.asarray(inputs["b2"], np.float32)

    prep = _prepare(cfg, edge_index)
    nc = _build(cfg, prep["chunk_blocks"], prep["calls"], debug=(mode == "sim"), taps=taps)
    in_maps = _make_in_maps(cfg, prep, x, W1, b1, W2, b2)

    info = {}
    if mode == "sim":
        from concourse.bass_interp import MultiCoreSim

        sim = MultiCoreSim(nc, cfg.CORES)
        for c in range(cfg.CORES):
            for k, v in in_maps[c].items():
                sim.cores[c].tensor(k)[:] = v
        sim.simulate()
        outs = [sim.cores[c].tensor("outT").copy() for c in range(cfg.CORES)]
    else:
        import concourse.bass_utils as bu

        if trace:
            # avoid the S3 artifact upload in the profile path
            bu.upload_artifacts = lambda d: "(local)"
        r = bu.run_bass_kernel_spmd(
            nc, in_maps, list(range(cfg.CORES)), trace=trace,
            tmpdir=(inputs.get("_tracedir") if trace else None),
        )
        info["exec_time_ns"] = r.exec_time_ns
        info["mean_exec_time_ns"] = r.mean_exec_time_ns
        outs = [r.results[c]["outT"] for c in range(cfg.CORES)]

    out = np.concatenate([o[:, : cfg.NPC].T for o in outs], axis=0)
    return out.astype(np.float32), info


def kernel(**inputs):
    out, _ = _run(Cfg(), inputs, mode="hw")
    return out

